# revision 35
# baseline (speedup 1.0000x reference)
"""Trainium2 Bass kernel for nn_CNN_25744033972549.

The reference network is three *linear* stages (conv k=10 pad=9, conv k=20
pad=19, sliding-window FC k=10 with edge-replicated left pad) with no
nonlinearity between them, applied causally.  The whole map is therefore a
single 38-tap causal conv  out[t] = B + sum_e E[e] @ x[t-e]  (zero-extended
x) plus closed-form boundary corrections for t < 28:

  out[t] += D[t] + [t < 9] * Q[t] @ (G0 @ x[b, 0] - P2_19)

where E, B, D, Q, G0, P2_19 are composed from (w1,b1,w2,b2,wf,bf) on the
host in float64.  This cuts device FLOPs ~100x vs running the three convs.

Sharding: data-parallel over batch, one batch element per NeuronCore
(B=8 = n_cores), weights replicated, no collectives.

Shared device layout (per core):
  xS (128, 4132): host-built, xS[32g+c, 36+tau] = x[b, tau-g, c]
    (4 tap-shifted copies of channel-major x; zero padding built in).
  ew (128, 240) : 10 K-chunk weight tiles, ew[32g+c, 24j+o] = E[4j+g,o,c].

Variants:
  a_f32  — fp32, time-major out (T,24): per 128-t tile, 10 accumulating
           matmuls with the x-window stationary (128 cols -> LDW-bound).
  b_f32r / b_bf16 — channel-major out (24,T), weights stationary (24 cols,
           ~20ns LDW), xS moving at 1 cyc/row: ~3x less PE time. Host
           transposes the (24,T) per-core outputs at gather time.
  c*     — current best family (~12.6x over b3_fp16): time-strided
           columns. xs4[32g+c, v] = x[4(v-9)-g, c] packs 4 tap-shifts in
           the 128 contraction rows AND strides columns by 4 timesteps,
           so the stationary weight grows to 96 wide (24 outputs x 4
           time-offsets d, W_j[32g+c,24d+o] = E[d+g+4j-4,o,c]) and one
           pass of NJ=11 chunks x U=1024 columns covers the whole conv:
           22 LDW+MM pairs of 512 rows/iter vs 232 (10.5x less PE work,
           75% PE-column utilization). Plain fp16 (no error
           compensation; gate is 2e-2, fp16 gives 4e-4); bias +
           boundary corrections applied on host at gather. Sub-variants:
           accumulation groups must be SEQUENTIAL per PSUM bank
           (interleaving them was 36x slower), and the repeat loop is
           unrolled u bodies per For_i iteration (cu/cf/cs{u}) to
           amortize For_i's per-iteration all-engine barrier (~5us) and
           keep the PE HAM clock-gate warm; u=16 is the sweet spot
           (u=32 thrashes instruction fetch). cf* pads weights to 128
           cols for Fast Weight Load; cs* adds staggered_reset (both
           ~noise-level vs cu16 once LDW pull-ahead hides the load).
  d*     — stride-5 evolution of c: since each chunk's 4 row-shifts can
           be ARBITRARY host-built offsets (chunk k carries taps
           {4k-4..4k-1} via its own pre-shifted x image), the column
           stride rises to 5 and the stationary packs 24 outputs x 5
           offsets = 120/128 PE columns. Same 22 LDW+MM pairs, but
           only 11 chunks x 820 columns = 9020 streamed columns
           (-20% PE time vs c). 11 images cost 18KB/partition SBUF.
"""

import os

import numpy as np

B, T, CIN, H, C2, O = 8, 4096, 32, 256, 512, 24
K1, K2, KF = 10, 20, 10
NE = 38          # composed conv taps
NCHUNK = 10      # ceil(NE/4) K-chunks of 128 = 4 taps x 32 channels
OFF = 36         # left halo lookback
W = OFF + T      # xS width
TILE = 128       # variant a: timesteps per tile
NTILES = T // TILE
TILE_B = 512     # variant b: timesteps per tile (one PSUM bank)
NTILES_B = T // TILE_B
NCORES = 8

# variant c: 4 time-offsets x 24 outputs = 96 PE columns per streamed column
NJ = 11          # weight chunks (tap offsets e-d in [-4, 39] cover [-3, 37])
FREE = 4 * O     # stationary free width
U = T // 4       # strided columns
WC = U + 10      # xs image width (10-col left halo)

# variant d: column stride 5 -> 5 time-offsets x 24 outputs = 120 PE
# columns; each of the 11 chunks gets its own 4-shift x image, so total
# streamed columns drop to 11*820 = 9020 (-20% vs variant c's 11264).
NK = 11          # chunks; chunk k rows carry tap offsets {4k-4 .. 4k-1}
UD = 820         # ceil(T/5) columns per image
FREE_D = 5 * O   # 120 used stationary columns (128 padded for FWL)

VARIANT = os.environ.get("KERNEL_VARIANT", "cfb16")

_cache = {}


def _compose(w1, b1, w2, b2, wf, bf):
    """Compose the three linear stages in float64. Returns
    (E (38,O,CIN), Bconst (O,), D (28,O), Q (9,O,C2), G0 (C2,CIN), P219 (C2,))."""
    w1 = w1.astype(np.float64)
    b1 = b1.astype(np.float64)
    w2 = w2.astype(np.float64)
    b2 = b2.astype(np.float64)
    wf = wf.astype(np.float64)
    bf = bf.astype(np.float64)
    WFk = wf.reshape(O, KF, C2)

    G = np.zeros((29, C2, CIN))
    for k1 in range(K1):
        for k2 in range(K2):
            G[28 - k1 - k2] += w2[:, :, k2] @ w1[:, :, k1]

    E = np.zeros((NE, O, CIN))
    for k in range(KF):
        for d in range(29):
            E[9 - k + d] += WFk[:, k, :] @ G[d]

    hbar = b2 + w2.sum(axis=2) @ b1
    Bconst = bf + WFk.sum(axis=1) @ hbar

    P2 = np.zeros((21, C2))
    for m in range(1, 21):
        P2[m] = P2[m - 1] + w2[:, :, m - 1] @ b1

    D = np.zeros((28, O))
    for t in range(28):
        for k in range(KF):
            j = t - 9 + k
            if 0 <= j < 19:
                D[t] -= WFk[:, k, :] @ P2[19 - j]

    Q = np.zeros((9, O, C2))
    for t in range(9):
        Q[t] = WFk[:, : 9 - t, :].sum(axis=1)

    return E, Bconst, D, Q, G[0], P2[19]


def _np_dtype(variant):
    if variant.endswith("bf16"):
        import ml_dtypes

        return np.dtype(ml_dtypes.bfloat16)
    if variant.endswith("fp16"):
        return np.dtype(np.float16)
    return np.dtype(np.float32)


def _build_program(variant=VARIANT, repeat=1):
    import concourse.bacc as bacc
    import concourse.mybir as mybir
    from concourse.tile import TileContext

    if variant.startswith("d"):
        return _build_program_d(repeat, variant)
    if variant.startswith("c"):
        return _build_program_c(repeat, variant)

    f32 = mybir.dt.float32
    if variant in ("a_f32", "m4_f32"):
        mmdt = f32
    elif variant == "b_f32r":
        mmdt = mybir.dt.float32r
    elif variant == "b_bf16":
        mmdt = mybir.dt.bfloat16
    elif variant in ("b_fp16", "b3_fp16"):
        mmdt = mybir.dt.float16
    else:
        raise ValueError(variant)

    nc = bacc.Bacc("TRN2", target_bir_lowering=False, debug=False)
    xs = nc.declare_dram_parameter("xs", [128, W], mmdt, isOutput=False)
    ew = nc.declare_dram_parameter("ew", [128, NCHUNK * O], mmdt, isOutput=False)

    with TileContext(nc) as tc:
        with (
            tc.tile_pool(name="const", bufs=1) as cpool,
            tc.tile_pool(name="xwp", bufs=4) as xpool,
            tc.tile_pool(name="ps", bufs=8, space="PSUM") as pspool,
            tc.tile_pool(name="ot", bufs=4) as opool,
        ):
            ew_sb = cpool.tile([128, NCHUNK * O], mmdt)
            nc.sync.dma_start(out=ew_sb[:, :], in_=ew[:, :])
            if variant != "a_f32":
                # whole shifted-x image stays resident in SBUF (1-2 MB)
                xs_sb = cpool.tile([128, W], mmdt)
                nc.sync.dma_start(out=xs_sb[:, :], in_=xs[:, :])
            if variant == "b3_fp16":
                # 2^10-scaled fp16 residuals of x and E for the
                # error-compensation passes
                xs2 = nc.declare_dram_parameter("xs2", [128, W], mmdt,
                                                isOutput=False)
                ew2 = nc.declare_dram_parameter("ew2", [128, NCHUNK * O], mmdt,
                                                isOutput=False)
                xs2_sb = cpool.tile([128, W], mmdt)
                nc.sync.dma_start(out=xs2_sb[:, :], in_=xs2[:, :])
                ew2_sb = cpool.tile([128, NCHUNK * O], mmdt)
                nc.sync.dma_start(out=ew2_sb[:, :], in_=ew2[:, :])
                # merged tail: rows 0-63 = E1 taps 36-37 vs x2,
                # rows 64-127 = E2 taps 36-37 vs x1 (one MM instead of two)
                xsc = nc.declare_dram_parameter("xsc", [128, W], mmdt,
                                                isOutput=False)
                ewc = nc.declare_dram_parameter("ewc", [128, O], mmdt,
                                                isOutput=False)
                xsc_sb = cpool.tile([128, W], mmdt)
                nc.sync.dma_start(out=xsc_sb[:, :], in_=xsc[:, :])
                ewc_sb = cpool.tile([128, O], mmdt)
                nc.sync.dma_start(out=ewc_sb[:, :], in_=ewc[:, :])

            if variant == "a_f32":
                # LDWEIGHTS from a wide resident tile measured 2.4x slower, so
                # stage compact per-tile windows via DMA instead.
                bias = nc.declare_dram_parameter("bias", [2 * 128, O], f32,
                                                 isOutput=False)
                out = nc.declare_dram_parameter("out", [T, O], f32, isOutput=True)
                bias0 = cpool.tile([128, O], f32)
                biasR = cpool.tile([128, O], f32)
                nc.sync.dma_start(out=bias0[:, :], in_=bias[0:128, :])
                nc.sync.dma_start(out=biasR[:, :], in_=bias[128:256, :])

                def body():
                    for i in range(NTILES):
                        t0 = i * TILE
                        xw = xpool.tile([128, OFF + TILE], f32)
                        nc.sync.dma_start(
                            out=xw[:, :], in_=xs[:, t0 : t0 + OFF + TILE]
                        )
                        ps = pspool.tile([128, O], f32, bufs=4)
                        for j in range(NCHUNK):
                            lo = OFF - 4 * j
                            nc.tensor.matmul(
                                out=ps[:, :],
                                lhsT=xw[:, lo : lo + 128],
                                rhs=ew_sb[:, j * O : (j + 1) * O],
                                start=(j == 0),
                                stop=(j == NCHUNK - 1),
                            )
                        ot = opool.tile([128, O], f32)
                        nc.vector.tensor_add(
                            out=ot[:, :],
                            in0=ps[:, :],
                            in1=(bias0 if i == 0 else biasR)[:, :],
                        )
                        nc.sync.dma_start(out=out[t0 : t0 + TILE, :], in_=ot[:, :])

            elif variant == "m4_f32":
                # fp32-exact, 4x column-tiled: 4 weight chunks stream
                # concurrently in disjoint 32-col PE strips; partials land in
                # 4 partition strips of one PSUM bank; a stacked-identity
                # fp32 matmul reduces the strips. out channel-major (24, T).
                bias = nc.declare_dram_parameter("bias", [2 * O, TILE_B], f32,
                                                 isOutput=False)
                idr = nc.declare_dram_parameter("idr", [128, O], f32,
                                                isOutput=False)
                out = nc.declare_dram_parameter("out", [O, T], f32, isOutput=True)
                bias0 = cpool.tile([O, TILE_B], f32)
                biasR = cpool.tile([O, TILE_B], f32)
                nc.sync.dma_start(out=bias0[:, :], in_=bias[0:O, :])
                nc.sync.dma_start(out=biasR[:, :], in_=bias[O : 2 * O, :])
                idr_sb = cpool.tile([128, O], f32)
                nc.sync.dma_start(out=idr_sb[:, :], in_=idr[:, :])
                # staging tile for PSUM->SBUF strip copies; zeroed once so the
                # 8-row bands between strips stay 0 for the reduce matmul
                cp = cpool.tile([128, TILE_B], f32)
                nc.any.memset(cp[:, :], 0.0)

                def body():
                    for i in range(NTILES_B):
                        t0 = i * TILE_B
                        ps = pspool.tile([128, TILE_B], f32, tag="psbank", bufs=3)
                        # waves: (j=0..3 on strips 0..3), (4..7), (8..9)
                        for g in range(3):
                            strips = range(4) if g < 2 else range(2)
                            for s in strips:
                                j = 4 * g + s
                                lo = t0 + OFF - 4 * j
                                nc.tensor.matmul(
                                    out=ps[32 * s : 32 * s + O, :],
                                    lhsT=ew_sb[:, j * O : (j + 1) * O],
                                    rhs=xs_sb[:, lo : lo + TILE_B],
                                    start=(g == 0),
                                    stop=(g == 2) or (g == 1 and s >= 2),
                                    tile_position=(0, 32 * s),
                                    skip_group_check=True,
                                )
                        for s in range(4):
                            nc.vector.tensor_copy(
                                out=cp[32 * s : 32 * s + O, :],
                                in_=ps[32 * s : 32 * s + O, :],
                            )
                        ps2 = pspool.tile([O, TILE_B], f32, tag="psred", bufs=3)
                        nc.tensor.matmul(
                            out=ps2[:, :], lhsT=idr_sb[:, :], rhs=cp[:, :],
                            start=True, stop=True,
                        )
                        ot = opool.tile([O, TILE_B], f32)
                        nc.vector.tensor_add(
                            out=ot[:, :],
                            in0=ps2[:, :],
                            in1=(bias0 if i == 0 else biasR)[:, :],
                        )
                        nc.sync.dma_start(
                            out=out[:, t0 : t0 + TILE_B], in_=ot[:, :]
                        )

            elif variant == "b3_fp16":
                # error-compensated fp16: out = E1*x1 + 2^-10 (E1*x2 + E2*x1)
                # with x2/E2 the 2^10-scaled fp16 residuals -> fp32-grade
                # accuracy on the fast 1-cyc/row path.
                bias = nc.declare_dram_parameter("bias", [2 * O, TILE_B], f32,
                                                 isOutput=False)
                out = nc.declare_dram_parameter("out", [O, T], f32, isOutput=True)
                bias0 = cpool.tile([O, TILE_B], f32)
                biasR = cpool.tile([O, TILE_B], f32)
                nc.sync.dma_start(out=bias0[:, :], in_=bias[0:O, :])
                nc.sync.dma_start(out=biasR[:, :], in_=bias[O : 2 * O, :])

                def body():
                    import concourse.mybir as mb

                    for i in range(NTILES_B):
                        t0 = i * TILE_B
                        psm = pspool.tile([O, TILE_B], f32, tag="psm", bufs=4)
                        for j in range(NCHUNK):
                            lo = t0 + OFF - 4 * j
                            nc.tensor.matmul(
                                out=psm[:, :],
                                lhsT=ew_sb[:, j * O : (j + 1) * O],
                                rhs=xs_sb[:, lo : lo + TILE_B],
                                start=(j == 0),
                                stop=(j == NCHUNK - 1),
                            )
                        psc = pspool.tile([O, TILE_B], f32, tag="psc", bufs=4)
                        for w, (esrc, xsrc) in enumerate(
                            ((ew_sb, xs2_sb), (ew2_sb, xs_sb))
                        ):
                            for j in range(NCHUNK - 1):
                                lo = t0 + OFF - 4 * j
                                nc.tensor.matmul(
                                    out=psc[:, :],
                                    lhsT=esrc[:, j * O : (j + 1) * O],
                                    rhs=xsrc[:, lo : lo + TILE_B],
                                    start=(w == 0 and j == 0),
                                    stop=False,
                                )
                        lo9 = t0 + OFF - 4 * (NCHUNK - 1)
                        nc.tensor.matmul(
                            out=psc[:, :],
                            lhsT=ewc_sb[:, :],
                            rhs=xsc_sb[:, lo9 : lo9 + TILE_B],
                            start=False,
                            stop=True,
                        )
                        # corr*2^-10 on ACT, then main + bias and sum on DVE
                        cr = opool.tile([O, TILE_B], f32, tag="cr", bufs=4)
                        nc.scalar.activation(
                            cr[:, :], psc[:, :],
                            mb.ActivationFunctionType.Copy,
                            scale=float(2.0 ** -10),
                        )
                        mb_ = opool.tile([O, TILE_B], f32, tag="mb", bufs=4)
                        nc.vector.tensor_add(
                            out=mb_[:, :],
                            in0=psm[:, :],
                            in1=(bias0 if i == 0 else biasR)[:, :],
                        )
                        ot = opool.tile([O, TILE_B], f32)
                        nc.vector.tensor_add(
                            out=ot[:, :], in0=mb_[:, :], in1=cr[:, :]
                        )
                        nc.sync.dma_start(
                            out=out[:, t0 : t0 + TILE_B], in_=ot[:, :]
                        )

            else:
                # channel-major: out_cm (24, T); bias blocks (24, TILE_B) x2
                bias = nc.declare_dram_parameter("bias", [2 * O, TILE_B], f32,
                                                 isOutput=False)
                out = nc.declare_dram_parameter("out", [O, T], f32, isOutput=True)
                bias0 = cpool.tile([O, TILE_B], f32)
                biasR = cpool.tile([O, TILE_B], f32)
                nc.sync.dma_start(out=bias0[:, :], in_=bias[0:O, :])
                nc.sync.dma_start(out=biasR[:, :], in_=bias[O : 2 * O, :])

                def body():
                    for i in range(NTILES_B):
                        t0 = i * TILE_B
                        ps = pspool.tile([O, TILE_B], f32)
                        for j in range(NCHUNK):
                            lo = t0 + OFF - 4 * j
                            nc.tensor.matmul(
                                out=ps[:, :],
                                lhsT=ew_sb[:, j * O : (j + 1) * O],
                                rhs=xs_sb[:, lo : lo + TILE_B],
                                start=(j == 0),
                                stop=(j == NCHUNK - 1),
                            )
                        ot = opool.tile([O, TILE_B], f32)
                        nc.vector.tensor_add(
                            out=ot[:, :],
                            in0=ps[:, :],
                            in1=(bias0 if i == 0 else biasR)[:, :],
                        )
                        nc.sync.dma_start(
                            out=out[:, t0 : t0 + TILE_B], in_=ot[:, :]
                        )

            if repeat == 1:
                body()
            else:
                hints = (
                    mybir.EngineType.PE,
                    mybir.EngineType.SP,
                    mybir.EngineType.DVE,
                    mybir.EngineType.Activation,
                )
                with tc.For_i(0, repeat, 1, hint_engines=hints):
                    body()
    nc.compile()
    return nc


def _build_program_c(repeat=1, variant="c_fp16"):
    """Variant c_fp16: time-strided columns. Each streamed rhs column u holds
    4 tap-shifted channel blocks of x at base time 4u; the stationary weight
    is 96 wide (4 time-offsets d x 24 outputs), so one pass of NJ=11 chunks
    over U=1024 columns covers the whole 38-tap conv: 22 matmuls of 512 rows
    vs 232 in the b variants. Bias + boundary corrections applied on host.

    Diagnostic sub-variants: cseq (banks sequential, not interleaved),
    chalf (one bank only; wrong output, timing probe), cnodma (tiny DMA
    out; wrong output, timing probe)."""
    import concourse.bacc as bacc
    import concourse.mybir as mybir
    from concourse.tile import TileContext

    f32 = mybir.dt.float32
    fp16 = mybir.dt.float16
    # cf*: weights zero-padded to 128 columns to trigger the compiler's
    # Fast Weight Load (requires exactly 128 weight cols); PSUM rows
    # 96..127 accumulate zeros and are never read.
    FW = 128 if variant.startswith("cf") else FREE

    nc = bacc.Bacc("TRN2", target_bir_lowering=False, debug=False)
    xs = nc.declare_dram_parameter("xs", [128, WC], fp16, isOutput=False)
    ew = nc.declare_dram_parameter("ew", [128, NJ * FW], fp16, isOutput=False)
    out = nc.declare_dram_parameter("out", [FREE, U], fp16, isOutput=True)

    with TileContext(nc) as tc:
        with (
            tc.tile_pool(name="const", bufs=1) as cpool,
            tc.tile_pool(name="ps", bufs=1, space="PSUM") as pspool,
            tc.tile_pool(name="ot", bufs=1) as opool,
        ):
            ew_sb = cpool.tile([128, NJ * FW], fp16)
            nc.sync.dma_start(out=ew_sb[:, :], in_=ew[:, :])
            xs_sb = cpool.tile([128, WC], fp16)
            nc.sync.dma_start(out=xs_sb[:, :], in_=xs[:, :])

            def mm_tile(ps, j, base):
                lo = base + 10 - j
                nc.tensor.matmul(
                    out=ps[:, :],
                    lhsT=ew_sb[:, j * FW : (j + 1) * FW],
                    rhs=xs_sb[:, lo : lo + 512],
                    start=(j == 0),
                    stop=(j == NJ - 1),
                )

            nps = 4 if "b" in variant else 2

            def body():
                ps0 = pspool.tile([FW, 512], f32, tag="ps0", bufs=nps)
                ot0 = opool.tile([FREE, 512], fp16, tag="o0", bufs=2)
                if variant == "chalf":
                    for j in range(NJ):
                        mm_tile(ps0, j, 0)
                    nc.scalar.activation(
                        ot0[:, :], ps0[0:FREE, :],
                        mybir.ActivationFunctionType.Copy,
                    )
                    nc.sync.dma_start(out=out[:, 0:512], in_=ot0[:, :])
                    return
                ps1 = pspool.tile([FW, 512], f32, tag="ps1", bufs=nps)
                ot1 = opool.tile([FREE, 512], fp16, tag="o1", bufs=2)
                if variant == "c_fp16":
                    for j in range(NJ):
                        mm_tile(ps0, j, 0)
                        mm_tile(ps1, j, 512)
                else:  # cseq / cu* / cf*: sequential accumulation groups
                    for j in range(NJ):
                        mm_tile(ps0, j, 0)
                    for j in range(NJ):
                        mm_tile(ps1, j, 512)
                nc.scalar.activation(
                    ot0[:, :], ps0[0:FREE, :],
                    mybir.ActivationFunctionType.Copy,
                )
                nc.vector.tensor_copy(out=ot1[:, :], in_=ps1[0:FREE, :])
                tiny = "n" in variant          # cn*/cnodma: timing probe
                qsplit = "q" in variant and variant != "cseq"
                if tiny:
                    nc.sync.dma_start(out=out[:, 0:16], in_=ot0[:, 0:16])
                    nc.sync.dma_start(out=out[:, 512:528], in_=ot1[:, 0:16])
                elif qsplit:
                    for q in range(2):
                        nc.sync.dma_start(
                            out=out[:, 256 * q : 256 * (q + 1)],
                            in_=ot0[:, 256 * q : 256 * (q + 1)],
                        )
                        nc.sync.dma_start(
                            out=out[:, 512 + 256 * q : 512 + 256 * (q + 1)],
                            in_=ot1[:, 256 * q : 256 * (q + 1)],
                        )
                else:
                    nc.sync.dma_start(out=out[:, 0:512], in_=ot0[:, :])
                    nc.sync.dma_start(out=out[:, 512:1024], in_=ot1[:, :])

            if repeat == 1:
                body()
            else:
                # unroll u bodies per For_i iteration: For_i has an
                # InstAllEngineBarrier + semaphore reset per iteration
                # (~5us fixed + HAM re-throttle); amortize it.
                digs = "".join(ch for ch in variant if ch.isdigit())
                u = (
                    int(digs)
                    if variant.startswith(("cu", "cf", "cs", "cn", "cq", "cb"))
                    and digs
                    else 1
                )
                while repeat % u:
                    u //= 2
                hints = (
                    mybir.EngineType.PE,
                    mybir.EngineType.SP,
                    mybir.EngineType.DVE,
                    mybir.EngineType.Activation,
                )
                stag = variant.startswith("cs") and variant != "cseq"
                with tc.For_i(
                    0, repeat // u, 1, hint_engines=hints,
                    staggered_reset=stag,
                ):
                    for _ in range(u):
                        body()
    nc.compile()
    return nc


def _build_program_d(repeat=1, variant="db16"):
    """Variant d: stride-5 columns, per-chunk pre-shifted x images.
    22 LDW+MM pairs as in variant c, but only 9020 streamed columns."""
    import concourse.bacc as bacc
    import concourse.mybir as mybir
    from concourse.tile import TileContext

    f32 = mybir.dt.float32
    fp16 = mybir.dt.float16
    W1 = 512            # first PSUM tile columns
    W2 = UD - W1        # second tile columns (308)

    nc = bacc.Bacc("TRN2", target_bir_lowering=False, debug=False)
    xs = nc.declare_dram_parameter("xs", [128, NK * UD], fp16, isOutput=False)
    ew = nc.declare_dram_parameter("ew", [128, NK * 128], fp16, isOutput=False)
    out = nc.declare_dram_parameter("out", [FREE_D, UD], fp16, isOutput=True)

    with TileContext(nc) as tc:
        with (
            tc.tile_pool(name="const", bufs=1) as cpool,
            tc.tile_pool(name="ps", bufs=1, space="PSUM") as pspool,
            tc.tile_pool(name="ot", bufs=1) as opool,
        ):
            ew_sb = cpool.tile([128, NK * 128], fp16)
            nc.sync.dma_start(out=ew_sb[:, :], in_=ew[:, :])
            xs_sb = cpool.tile([128, NK * UD], fp16)
            nc.sync.dma_start(out=xs_sb[:, :], in_=xs[:, :])

            nps = 4 if "b" in variant else 2

            def body():
                ps0 = pspool.tile([128, W1], f32, tag="ps0", bufs=nps)
                ps1 = pspool.tile([128, W2], f32, tag="ps1", bufs=nps)
                for k in range(NK):
                    nc.tensor.matmul(
                        out=ps0[:, :],
                        lhsT=ew_sb[:, k * 128 : (k + 1) * 128],
                        rhs=xs_sb[:, k * UD : k * UD + W1],
                        start=(k == 0),
                        stop=(k == NK - 1),
                    )
                for k in range(NK):
                    nc.tensor.matmul(
                        out=ps1[:, :],
                        lhsT=ew_sb[:, k * 128 : (k + 1) * 128],
                        rhs=xs_sb[:, k * UD + W1 : (k + 1) * UD],
                        start=(k == 0),
                        stop=(k == NK - 1),
                    )
                ot0 = opool.tile([FREE_D, W1], fp16, tag="o0", bufs=2)
                ot1 = opool.tile([FREE_D, W2], fp16, tag="o1", bufs=2)
                nc.scalar.activation(
                    ot0[:, :], ps0[0:FREE_D, :],
                    mybir.ActivationFunctionType.Copy,
                )
                nc.vector.tensor_copy(out=ot1[:, :], in_=ps1[0:FREE_D, :])
                nc.sync.dma_start(out=out[:, 0:W1], in_=ot0[:, :])
                nc.sync.dma_start(out=out[:, W1:UD], in_=ot1[:, :])

            if repeat == 1:
                body()
            else:
                digs = "".join(ch for ch in variant if ch.isdigit())
                u = int(digs) if digs else 1
                while repeat % u:
                    u //= 2
                hints = (
                    mybir.EngineType.PE,
                    mybir.EngineType.SP,
                    mybir.EngineType.DVE,
                    mybir.EngineType.Activation,
                )
                with tc.For_i(0, repeat // u, 1, hint_engines=hints):
                    for _ in range(u):
                        body()
    nc.compile()
    return nc


_post_ctx = {}


def _prep_d(inputs):
    x = np.ascontiguousarray(np.asarray(inputs["x"], dtype=np.float32))
    E, Bconst, D, Q, G0, P219 = _compose(
        np.asarray(inputs["w1"]), np.asarray(inputs["b1"]),
        np.asarray(inputs["w2"]), np.asarray(inputs["b2"]),
        np.asarray(inputs["wf"]), np.asarray(inputs["bf"]),
    )

    ewd = np.zeros((128, NK * 128))
    for k in range(NK):
        for g in range(4):
            sig = 4 * k - 4 + g
            for d in range(5):
                e = d + sig
                if 0 <= e < NE:
                    ewd[32 * g : 32 * g + 32,
                        128 * k + O * d : 128 * k + O * (d + 1)] = E[e].T
    ewd = np.ascontiguousarray(ewd.astype(np.float16))

    xsd = np.zeros((B, 128, NK * UD), dtype=np.float16)
    xT = x.transpose(0, 2, 1)
    v = np.arange(UD)
    for k in range(NK):
        for g in range(4):
            idx = 5 * v - (4 * k - 4 + g)
            ok = (idx >= 0) & (idx < T)
            xsd[:, 32 * g : 32 * g + 32, UD * k + np.nonzero(ok)[0]] = (
                xT[:, :, idx[ok]].astype(np.float16)
            )

    corrD = np.zeros((B, 28, O))
    for b in range(B):
        vb = G0 @ x[b, 0].astype(np.float64) - P219
        corrD[b] = D
        corrD[b, :9] += Q @ vb
    _post_ctx["c"] = (Bconst, corrD)

    return [{"xs": np.ascontiguousarray(xsd[b]), "ew": ewd} for b in range(B)]


def _gather_d(results):
    Bconst, corrD = _post_ctx["c"]
    out2 = np.stack(
        [np.asarray(results[b]["out"]).astype(np.float32) for b in range(B)]
    )
    out = (out2.reshape(B, 5, O, UD).transpose(0, 3, 1, 2)
           .reshape(B, 5 * UD, O)[:, :T])
    out = out + Bconst[None, None, :].astype(np.float32)
    out[:, :28, :] += corrD.astype(np.float32)
    return np.ascontiguousarray(out.astype(np.float32))


def _prep_c(inputs, wide=False):
    x = np.ascontiguousarray(np.asarray(inputs["x"], dtype=np.float32))
    E, Bconst, D, Q, G0, P219 = _compose(
        np.asarray(inputs["w1"]), np.asarray(inputs["b1"]),
        np.asarray(inputs["w2"]), np.asarray(inputs["b2"]),
        np.asarray(inputs["wf"]), np.asarray(inputs["bf"]),
    )

    FW = 128 if wide else FREE
    ewc = np.zeros((128, NJ * FW))
    for j in range(NJ):
        for g in range(4):
            for d in range(4):
                e = d + g + 4 * j - 4
                if 0 <= e < NE:
                    ewc[32 * g : 32 * g + 32,
                        FW * j + O * d : FW * j + O * (d + 1)] = E[e].T
    ewc = np.ascontiguousarray(ewc.astype(np.float16))

    xs4 = np.zeros((B, 128, WC), dtype=np.float16)
    xT = x.transpose(0, 2, 1)
    v = np.arange(WC)
    for g in range(4):
        idx = 4 * (v - 9) - g
        valid = (idx >= 0) & (idx < T)
        xs4[:, 32 * g : 32 * g + 32, valid] = xT[:, :, idx[valid]].astype(
            np.float16
        )

    corrD = np.zeros((B, 28, O))
    for b in range(B):
        vb = G0 @ x[b, 0].astype(np.float64) - P219
        corrD[b] = D
        corrD[b, :9] += Q @ vb
    _post_ctx["c"] = (Bconst, corrD)

    return [{"xs": np.ascontiguousarray(xs4[b]), "ew": ewc} for b in range(B)]


def _gather_c(results):
    Bconst, corrD = _post_ctx["c"]
    out2 = np.stack(
        [np.asarray(results[b]["out"]).astype(np.float32) for b in range(B)]
    )
    out = out2.reshape(B, 4, O, U).transpose(0, 3, 1, 2).reshape(B, T, O)
    out = out + Bconst[None, None, :].astype(np.float32)
    out[:, :28, :] += corrD.astype(np.float32)
    return np.ascontiguousarray(out.astype(np.float32))


def _flush16(a):
    """Cast to fp16, flushing denormals to zero (PE may FTZ; the host must
    match so the residual pass captures the flushed part)."""
    h = a.astype(np.float16)
    h[np.abs(h.astype(np.float32)) < 2.0 ** -14] = np.float16(0)
    return h


def _layout_ew(Epad, ndt):
    """(40, O, CIN) -> (128, 240): ew[32g + c, 24j + o] = Epad[4j+g, o, c],
    the on-chip layout, so a single contiguous DMA loads it."""
    return np.ascontiguousarray(
        np.asarray(Epad, dtype=np.float64)
        .reshape(NCHUNK, 4, O, CIN)              # (j, g, o, c)
        .transpose(1, 3, 0, 2)                   # (g, c, j, o)
        .reshape(128, NCHUNK * O)
        .astype(ndt)
    )


def _layout_xs(x, ndt):
    """(B, T, CIN) -> (B, 128, W): xS[b, 32g+c, OFF+g+r] = x[b, r, c]."""
    xS = np.zeros((B, 128, W), dtype=ndt)
    xT = np.asarray(x).transpose(0, 2, 1).astype(ndt)  # (B, CIN, T)
    for g in range(4):
        n = min(T, W - OFF - g)
        xS[:, 32 * g : 32 * g + 32, OFF + g : OFF + g + n] = xT[:, :, :n]
    return xS


def _prep_in_maps(inputs, variant=VARIANT):
    if variant.startswith("d"):
        return _prep_d(inputs)
    if variant.startswith("c"):
        return _prep_c(inputs, wide=variant.startswith("cf"))
    x = np.ascontiguousarray(np.asarray(inputs["x"], dtype=np.float32))
    E, Bconst, D, Q, G0, P219 = _compose(
        np.asarray(inputs["w1"]), np.asarray(inputs["b1"]),
        np.asarray(inputs["w2"]), np.asarray(inputs["b2"]),
        np.asarray(inputs["wf"]), np.asarray(inputs["bf"]),
    )
    ndt = _np_dtype(variant)

    Epad = np.zeros((40, O, CIN))
    Epad[:NE] = E

    if variant == "b3_fp16":
        E1 = _flush16(Epad)
        E2 = _flush16((Epad - E1.astype(np.float64)) * 2.0 ** 10)
        x1 = _flush16(x)
        x2 = _flush16((x.astype(np.float64) - x1.astype(np.float64)) * 2.0 ** 10)
        ew = _layout_ew(E1, ndt)
        ew2 = _layout_ew(E2, ndt)
        xS = _layout_xs(x1, ndt)
        xS2 = _layout_xs(x2, ndt)
    else:
        ew = _layout_ew(Epad, ndt)
        xS = _layout_xs(x, ndt)

    # per-core per-timestep bias (fp32): corr[t] for t < 28, else Bconst
    corr = np.zeros((B, 28, O))
    for b in range(B):
        v = G0 @ x[b, 0].astype(np.float64) - P219
        corr[b] = D + Bconst
        corr[b, :9] += Q @ v

    if variant == "a_f32":
        bias_all = np.empty((B, 2 * 128, O), dtype=np.float32)
        for b in range(B):
            bias_all[b] = np.broadcast_to(Bconst, (256, O))
            bias_all[b, :28] = corr[b]
    else:
        bias_all = np.empty((B, 2 * O, TILE_B), dtype=np.float32)
        for b in range(B):
            bias_all[b] = np.tile(Bconst[:, None], (2, TILE_B))
            bias_all[b, :O, :28] = corr[b].T

    maps = [
        {"xs": np.ascontiguousarray(xS[b]), "ew": ew,
         "bias": np.ascontiguousarray(bias_all[b])}
        for b in range(B)
    ]
    if variant == "m4_f32":
        idr = np.zeros((128, O), dtype=np.float32)
        for s in range(4):
            idr[32 * s + np.arange(O), np.arange(O)] = 1.0
        for m in maps:
            m["idr"] = idr
    if variant == "b3_fp16":
        ewc = np.ascontiguousarray(
            np.vstack([ew[0:64, (NCHUNK - 1) * O :],
                       ew2[0:64, (NCHUNK - 1) * O :]])
        )
        for b, m in enumerate(maps):
            m["xs2"] = np.ascontiguousarray(xS2[b])
            m["ew2"] = ew2
            m["xsc"] = np.ascontiguousarray(
                np.vstack([xS2[b][0:64], xS[b][0:64]])
            )
            m["ewc"] = ewc
    return maps


def _get_program(variant=VARIANT, repeat=1):
    key = (variant, repeat)
    if key not in _cache:
        _cache[key] = _build_program(variant, repeat)
    return _cache[key]


def _gather(results, variant=VARIANT):
    if variant.startswith("d"):
        return _gather_d(results)
    if variant.startswith("c"):
        return _gather_c(results)
    out = np.stack([np.asarray(results[b]["out"]) for b in range(B)])
    if variant != "a_f32":
        out = np.ascontiguousarray(out.transpose(0, 2, 1))
    return out.astype(np.float32, copy=False)


def _run(inputs, variant=VARIANT, trace=False, **spmd_kwargs):
    from concourse.bass_utils import run_bass_kernel_spmd

    nc = _get_program(variant)
    in_maps = _prep_in_maps(inputs, variant)
    res = run_bass_kernel_spmd(
        nc, in_maps, list(range(NCORES)), trace=trace, **spmd_kwargs
    )
    return _gather(res.results, variant), res


def kernel(**inputs) -> np.ndarray:
    out, _ = _run(inputs, trace=False)
    return out

